# revision 27
# baseline (speedup 1.0000x reference)
"""ChebyKAN layer kernel for 8 Trainium2 NeuronCores.

Reference computation:
    t = tanh(clip(x, -10, 10))                       # [N, I]
    ch = stack([T0(t) .. T4(t)], -1)                  # Chebyshev basis, deg 4
    out = clip(einsum('nid,oid->no', ch, coeffs), -10, 10)

Since t = tanh(.) lies in (-1, 1), every Chebyshev value is in [-1, 1] and the
intermediate clips at +-10 are no-ops.  We rewrite the basis as
{t, v, t*v, v^2} with v = 2t^2-1 (all values bounded by 1, so fp16 rounding
noise is not amplified) on the host:

    out[n,o] = bias[o] + sum_i ( (c1-c3) t + c2 v + 2 c3 (tv) + 2 c4 v^2 )
    bias[o] = sum_i (c0 - c4)[o,i]          (added on host)

which is a [N, 4*I] x [4*I, O] matmul after the elementwise basis prep.

Sharding over 8 cores: 4-way over N (rows of x), 2-way over O (output
columns).  Each core holds W-shard [K=8192, 1024] fp16 resident in SBUF and
streams 32 tiles of 128 rows of x, computing tanh + powers on
scalar/vector engines and the matmul on the tensor engine (fp16 operands,
fp32 PSUM accumulation).
"""

import numpy as np

N, I, O, DEG = 16384, 2048, 2048, 4
NB, OB = 4, 2                      # core grid: 4-way over N, 2-way over O
NSH = N // NB                      # 4096 rows per core
OSH = O // OB                      # 1024 out cols per core
NT = NSH // 128                    # 32 n-tiles per core
KT = 4 * (I // 128)                # 64 contraction tiles (4 powers x 16 i-blocks)
IB = I // 128                      # 16 i-blocks


def _build_program():
    from concourse.bacc import Bacc
    from concourse.tile import TileContext
    import concourse.mybir as mybir

    f32 = mybir.dt.float32
    f16 = mybir.dt.float16
    TANH = mybir.ActivationFunctionType.Tanh

    nc = Bacc(None, target_bir_lowering=False)
    xt_d = nc.dram_tensor("xt", [NT, 128, I], f32, kind="ExternalInput")
    wt_d = nc.dram_tensor("wt", [KT, 128, OSH], f16, kind="ExternalInput")
    out_d = nc.dram_tensor("out", [NT, 128, OSH], f32, kind="ExternalOutput")

    NWARM = 2  # n-tiles processed k-major-interleaved while W streams in

    with TileContext(nc) as tc:
        with (
            tc.tile_pool(name="wpool", bufs=1) as wpool,
            tc.tile_pool(name="work", bufs=2) as pool,
            tc.tile_pool(name="opool", bufs=1) as opool,
            tc.tile_pool(name="tpool", bufs=2) as tpool,
            tc.tile_pool(name="psum", bufs=8, space="PSUM") as pp,
        ):
            def load_powers_warm(nt):
                # Warmup variant: x DMA + tanh + t1 cast in two 1024-col
                # halves, t1 as two half tiles, everything on the SP queue
                # ahead of the W stream.  The first matmul only needs t1's
                # first 128 columns, so this moves the PE start ~6us earlier.
                xt = pool.tile([128, I], f32, tag="xt")
                t1h = []
                for c in range(2):
                    s = slice(c * 1024, (c + 1) * 1024)
                    nc.sync.dma_start(out=xt[:, s], in_=xt_d[nt, :, s])
                    nc.scalar.activation(xt[:, s], xt[:, s], TANH)
                    th = tpool.tile([128, 1024], f16, tag=f"t1h{c}")
                    nc.vector.tensor_copy(th[:], xt[:, s])
                    t1h.append(th)
                uv = pool.tile([128, I], f32, tag="uv")
                nc.vector.tensor_mul(uv[:], xt[:], xt[:])
                nc.vector.tensor_scalar(
                    uv[:], uv[:], 2.0, -1.0,
                    mybir.AluOpType.mult, mybir.AluOpType.add,
                )
                t2 = tpool.tile([128, I], f16, tag="t2")
                nc.vector.tensor_copy(t2[:], uv[:])
                t3 = tpool.tile([128, I], f16, tag="t3")
                nc.vector.tensor_mul(t3[:], xt[:], uv[:])
                t4 = tpool.tile([128, I], f16, tag="t4")
                nc.vector.tensor_mul(t4[:], uv[:], uv[:])
                return [t1h, t2, t3, t4]

            def lhs_slice_warm(tp, p, ib):
                if p == 0:
                    return tp[0][ib >> 3][:, (ib & 7) * 128:((ib & 7) + 1) * 128]
                return tp[p][:, ib * 128:(ib + 1) * 128]

            def load_powers(nt):
                # Basis planes {t, v, t*v, v^2} with v = 2t^2-1: all bounded
                # by 1, so fp16 rounding noise is not amplified by large
                # monomial weights.
                xt = pool.tile([128, I], f32, tag="xt")
                nc.sync.dma_start(out=xt[:], in_=xt_d[nt])
                # t = tanh(x), in place (fp32)
                nc.scalar.activation(xt[:], xt[:], TANH)
                t1 = tpool.tile([128, I], f16, tag="t1")
                nc.vector.tensor_copy(t1[:], xt[:])
                uv = pool.tile([128, I], f32, tag="uv")
                nc.vector.tensor_mul(uv[:], xt[:], xt[:])
                # v = 2u - 1, in place (fp32)
                nc.vector.tensor_scalar(
                    uv[:], uv[:], 2.0, -1.0,
                    mybir.AluOpType.mult, mybir.AluOpType.add,
                )
                t2 = tpool.tile([128, I], f16, tag="t2")
                nc.vector.tensor_copy(t2[:], uv[:])
                t3 = tpool.tile([128, I], f16, tag="t3")
                nc.vector.tensor_mul(t3[:], xt[:], uv[:])
                t4 = tpool.tile([128, I], f16, tag="t4")
                nc.vector.tensor_mul(t4[:], uv[:], uv[:])
                return [t1, t2, t3, t4]

            def lhs_slice(tp, p, ib):
                return tp[p][:, ib * 128:(ib + 1) * 128]

            def store_out_oc(nt, osb, oc):
                nc.sync.dma_start(
                    out=out_d[nt, :, oc * 512:(oc + 1) * 512],
                    in_=osb[:, oc * 512:(oc + 1) * 512],
                )

            # x tiles for the warmup n-tiles first so their DMAs aren't
            # queued behind the 16 MB of W.
            tps = []
            for wnt in range(NWARM):
                tp_w = load_powers_warm(wnt)
                tps.append(tp_w)

            wtiles = []
            for k in range(KT):
                w = wpool.tile([128, OSH], f16, tag=f"w{k}")
                nc.sync.dma_start(out=w[:], in_=wt_d[k])
                wtiles.append(w)

            # Warmup phase: k-major across NWARM*2 psum groups, so the PE has
            # work for every W k-tile as it lands instead of idling until the
            # whole W shard is resident.
            groups = [(nt, oc) for nt in range(NWARM) for oc in range(OSH // 512)]
            pss = {}
            for g in groups:
                ps_tile = pp.tile([128, 512], f32, tag="ps")
                pss[g] = ps_tile
            for k in range(KT):
                p, ib = divmod(k, IB)
                for (nt, oc) in groups:
                    nc.tensor.matmul(
                        pss[(nt, oc)][:],
                        lhs_slice_warm(tps[nt], p, ib),
                        wtiles[k][:, oc * 512:(oc + 1) * 512],
                        start=(k == 0),
                        stop=(k == KT - 1),
                    )
            for nt in range(NWARM):
                osb = opool.tile([128, OSH], f32, tag="osb")
                for oc in range(OSH // 512):
                    nc.scalar.copy(osb[:, oc * 512:(oc + 1) * 512], pss[(nt, oc)][:])
                    store_out_oc(nt, osb, oc)

            # Steady state: W fully resident, group-major.
            for nt in range(NWARM, NT):
                tp = load_powers(nt)
                osb = opool.tile([128, OSH], f32, tag="osb")
                for oc in range(OSH // 512):
                    ps = pp.tile([128, 512], f32, tag="ps")
                    for k in range(KT):
                        p, ib = divmod(k, IB)
                        nc.tensor.matmul(
                            ps[:],
                            lhs_slice(tp, p, ib),
                            wtiles[k][:, oc * 512:(oc + 1) * 512],
                            start=(k == 0),
                            stop=(k == KT - 1),
                        )
                    nc.scalar.copy(osb[:, oc * 512:(oc + 1) * 512], ps[:])
                    store_out_oc(nt, osb, oc)

    nc.finalize()
    return nc


def _prep_inputs(x, coeffs):
    """Host-side shard prep: transposed/tiled x per N-block, packed fp16
    weights per O-block, and the T0/bias term."""
    # Basis on device: {t, v, t*v, v^2} with v = 2t^2 - 1 (= T2).  Then
    # T1 = t, T2 = v, T3 = 2(tv) - t, T4 = 2v^2 - 1, so
    # out = (c0 - c4) + (c1 - c3) t + c2 v + 2 c3 (tv) + 2 c4 v^2.
    c = coeffs.astype(np.float64)
    w_mono = np.stack(
        [
            c[..., 1] - c[..., 3],    # t
            c[..., 2],                # v
            2.0 * c[..., 3],          # t*v
            2.0 * c[..., 4],          # v^2
        ]
    )  # [4, O, I]
    bias = (c[..., 0] - c[..., 4]).sum(axis=1)  # [O] float64

    xparts = []
    for nb in range(NB):
        xs = x[nb * NSH:(nb + 1) * NSH, :]                 # [NSH, I]
        # [nt, n_in, i_blk, i_in] -> [nt, i_in, i_blk, n_in]
        xp = xs.reshape(NT, 128, IB, 128).transpose(0, 3, 2, 1)
        xparts.append(np.ascontiguousarray(xp.reshape(NT, 128, I), dtype=np.float32))

    wparts = []
    for ob in range(OB):
        wsl = w_mono[:, ob * OSH:(ob + 1) * OSH, :]        # [4, OSH, I]
        # -> [p, i_blk, i_in, o]
        wp = wsl.transpose(0, 2, 1).reshape(4, IB, 128, OSH)
        wparts.append(np.ascontiguousarray(wp.reshape(KT, 128, OSH), dtype=np.float16))

    return xparts, wparts, bias


def _run(x, coeffs, trace=False):
    import os

    from concourse.bass_utils import run_bass_kernel_spmd

    if not trace:
        # A stray BASS_TRACE in the environment would route through the NTFF
        # profile hook, which this image does not ship.
        os.environ["BASS_NEVER_TRACE"] = "1"
    else:
        os.environ.pop("BASS_NEVER_TRACE", None)

    xparts, wparts, bias = _prep_inputs(x, coeffs)
    nc = _build_program()
    in_maps = [
        {"xt": xparts[c // OB], "wt": wparts[c % OB]} for c in range(NB * OB)
    ]
    res = run_bass_kernel_spmd(nc, in_maps, list(range(NB * OB)), trace=trace)

    out = np.empty((N, O), dtype=np.float64)
    for c in range(NB * OB):
        nb, ob = divmod(c, OB)
        out[nb * NSH:(nb + 1) * NSH, ob * OSH:(ob + 1) * OSH] = (
            res.results[c]["out"].reshape(NSH, OSH)
        )
    out += bias[None, :]
    np.clip(out, -10.0, 10.0, out=out)
    return out.astype(np.float32), res


def kernel(x, coeffs):
    return _run(np.asarray(x), np.asarray(coeffs))[0]
